# revision 9
# baseline (speedup 1.0000x reference)
"""Trainium2 Bass kernel for a 4-layer GPT-style transformer (B=2, S=1024,
D=512, H=8, DFF=2048, V=50257) sharded over 8 NeuronCores.

Sharding: token-sharded trunk.  Core c (b=c//4, j=c%4) owns tokens {t: t%4==j}
of batch b, as 2 q-tiles of 128: row r of q-tile g <-> token t = 512g + 4r + j.
Per layer: LN own tokens -> PE-transpose -> AllGather hT within the 4-core
batch group -> K/V for the whole batch (4x redundant, balanced), attention for
own queries (identical causal block structure on every core by construction),
FFN for own tokens.  lm_head is vocab-sharded over all 8 cores after a final
8-way AllGather; logits are written transposed [Vslice, 2048] and the host
reassembles.  All matmuls bf16 with fp32 PSUM accumulation; LN gains and the
1/sqrt(dk) scale are folded into weights host-side.
"""

import numpy as np
import ml_dtypes

import concourse.bass as bass
import concourse.mybir as mybir
import concourse.tile as tile
from concourse import bacc
from concourse.bass_utils import run_bass_kernel_spmd
from concourse.masks import make_identity

AF = mybir.ActivationFunctionType
Alu = mybir.AluOpType
f32 = mybir.dt.float32
bf16 = mybir.dt.bfloat16

V, D, H, DK, DFF, L, B, S = 50257, 512, 8, 64, 2048, 4, 2, 1024
NC, P = 8, 128
EPS = 1e-5
TOK = 256                    # tokens per core
NT = TOK // P                # q-tiles per core (2)
KD = D // P                  # dm tiles (4)
KF = DFF // P                # dff tiles (16)
VS_PAD = 6400                # per-core vocab slice, padded
VT_FULL = VS_PAD // P        # vocab tiles per core (50)
VS = [6283] * 7 + [V - 7 * 6283]   # valid vocab per core

_bf = lambda a: np.ascontiguousarray(np.asarray(a).astype(ml_dtypes.bfloat16))
_f32 = lambda a: np.ascontiguousarray(np.asarray(a, dtype=np.float32))


def build(n_layers=L, vt=VT_FULL, debug=False):
    nc = bacc.Bacc("TRN2", target_bir_lowering=False, debug=False, num_devices=NC)

    x0_in = nc.dram_tensor("x0", [TOK, D], f32, kind="ExternalInput")
    masks_in = nc.dram_tensor("masks", [4, P, P], bf16, kind="ExternalInput")
    Ws = []
    for l in range(n_layers):
        Ws.append({k: nc.dram_tensor(f"{k}{l}", shp, dt, kind="ExternalInput")
                   for k, shp, dt in [
                       ("wq", [D, D], bf16), ("wk", [D, D], bf16),
                       ("wv", [D, D], bf16), ("wo", [D, D], bf16),
                       ("w1", [D, DFF], bf16), ("w2", [DFF, D], bf16),
                       ("bq", [D], f32), ("bk", [D], f32), ("bv", [D], f32),
                       ("bo", [P, D], f32), ("b1", [DFF], f32),
                       ("b2", [P, D], f32)]})
    whead = nc.dram_tensor("whead", [D, vt * P], bf16, kind="ExternalInput")
    bhead = nc.dram_tensor("bhead", [vt * P], f32, kind="ExternalInput")
    logits_out = nc.dram_tensor("logitsT", [vt * P, B * S], f32, kind="ExternalOutput")
    xdbg = (nc.dram_tensor("xdbg", [n_layers, TOK, D], f32, kind="ExternalOutput")
            if debug else None)

    GROUPS_BATCH = [[0, 1, 2, 3], [4, 5, 6, 7]]
    GROUPS_ALL = [list(range(NC))]
    ags = []
    for l in range(n_layers):
        ai = nc.dram_tensor(f"agin{l}", [D, TOK], bf16)
        ao = nc.dram_tensor(f"agout{l}", [4 * D, TOK], bf16)
        ags.append((ai, ao))
    af_i = nc.dram_tensor("aginF", [D, TOK], bf16)
    af_o = nc.dram_tensor("agoutF", [NC * D, TOK], bf16, addr_space="Shared")

    import contextlib
    with tile.TileContext(nc) as tc, contextlib.ExitStack() as ctx:
        const = ctx.enter_context(tc.tile_pool(name="const", bufs=1))
        xp = ctx.enter_context(tc.tile_pool(name="xp", bufs=2))
        wp = ctx.enter_context(tc.tile_pool(name="wp", bufs=1))
        whp = ctx.enter_context(tc.tile_pool(name="whp", bufs=1))
        act = ctx.enter_context(tc.tile_pool(name="act", bufs=2))
        atn = ctx.enter_context(tc.tile_pool(name="atn", bufs=2))
        sm = ctx.enter_context(tc.tile_pool(name="sm", bufs=2))
        ps = ctx.enter_context(tc.tile_pool(name="ps", bufs=2, space="PSUM"))
        psu = ctx.enter_context(tc.tile_pool(name="psu", bufs=1, space="PSUM"))
        pss = ctx.enter_context(tc.tile_pool(name="pss", bufs=3, space="PSUM"))

        ident = const.tile([P, P], bf16)
        make_identity(nc, ident)
        ones_col = const.tile([P, 1], bf16)
        nc.vector.memset(ones_col, 1.0)
        eps_t = const.tile([P, 1], f32)
        nc.vector.memset(eps_t, EPS)
        masks = const.tile([P, 4, P], bf16)
        nc.sync.dma_start(out=masks, in_=bass.AP(
            tensor=masks_in, offset=0, ap=[[P, P], [P * P, 4], [1, P]]))

        x_t = [xp.tile([P, D], f32, tag=f"x{t}", name=f"x_{t}") for t in range(NT)]
        for t in range(NT):
            nc.sync.dma_start(out=x_t[t], in_=x0_in[t * P:(t + 1) * P, :])

        def layernorm(src_tiles, tag):
            out = []
            for t in range(NT):
                stats = sm.tile([P, 6], f32, tag="stats")
                nc.vector.bn_stats(stats, src_tiles[t])
                mv = sm.tile([P, 2], f32, tag="mv")
                nc.vector.bn_aggr(mv, stats)
                sd = sm.tile([P, 1], f32, tag="sd")
                nc.scalar.activation(sd, mv[:, 1:2], AF.Sqrt, bias=eps_t, scale=1.0)
                nc.vector.reciprocal(sd, sd)
                h = act.tile([P, D], bf16, tag=f"{tag}{t}")
                nc.vector.tensor_scalar(
                    out=h, in0=src_tiles[t], scalar1=mv[:, 0:1], scalar2=sd,
                    op0=Alu.subtract, op1=Alu.mult)
                out.append(h)
            return out

        def transpose_own(h_tiles, tag):
            hT = act.tile([P, KD, TOK], bf16, tag=tag)
            for d in range(KD):
                for t in range(NT):
                    pt = ps.tile([P, P], bf16, tag="tp", bufs=1)
                    nc.tensor.transpose(pt, h_tiles[t][:, d * P:(d + 1) * P], ident)
                    nc.scalar.copy(hT[:, d, t * P:(t + 1) * P], pt)
            return hT

        def bias_tile(vec_dram, n, tag):
            bt = sm.tile([P, n], f32, tag=tag)
            nc.sync.dma_start(out=bt, in_=bass.AP(
                tensor=vec_dram, offset=0, ap=[[1, P], [P, n]]))
            return bt

        def load_w(dram, kdim, ndim, tag):
            tiles = []
            for k in range(kdim):
                wt = wp.tile([P, ndim], bf16, tag=f"{tag}{k}")
                nc.sync.dma_start(out=wt, in_=dram[k * P:(k + 1) * P, :])
                tiles.append(wt)
            return tiles

        for l in range(n_layers):
            W = Ws[l]
            ai, ao = ags[l]

            # ---- LN1, transpose, AllGather (4-way, own batch) ----
            h1 = layernorm(x_t, "h1_")
            hT_own = transpose_own(h1, "hTown")
            nc.sync.dma_start(
                out=bass.AP(tensor=ai, offset=0,
                            ap=[[TOK, P], [P * TOK, KD], [1, TOK]]),
                in_=hT_own)
            nc.gpsimd.collective_compute(
                "AllGather", Alu.bypass, replica_groups=GROUPS_BATCH,
                ins=[ai.ap().opt()], outs=[ao.ap().opt()])

            wqT = load_w(W["wq"], KD, D, "wq")
            wkT = load_w(W["wk"], KD, D, "wk")
            wvT = load_w(W["wv"], KD, D, "wv")
            bq_t = bias_tile(W["bq"], KD, "bq")
            bk_t = bias_tile(W["bk"], KD, "bk")
            bv_t = bias_tile(W["bv"], KD, "bv")

            # ---- Q from own hT (overlaps the AllGather) ----
            qT = atn.tile([P, KD, TOK], bf16, tag="qT")
            for m in range(KD):
                pq = ps.tile([P, TOK], f32, tag="mm")
                for k in range(KD):
                    nc.tensor.matmul(pq, wqT[k][:, m * P:(m + 1) * P],
                                     hT_own[:, k, :],
                                     start=(k == 0), stop=(k == KD - 1))
                nc.scalar.activation(qT[:, m, :], pq, AF.Identity,
                                     bias=bq_t[:, m:m + 1], scale=1.0)

            # ---- gathered hT for the batch: [P, KD, 1024] ----
            hT_all = act.tile([P, KD, 4 * TOK], bf16, tag="hTall", bufs=1)
            for jp in range(4):
                for d in range(KD):
                    nc.sync.dma_start(
                        out=hT_all[:, d, jp * TOK:(jp + 1) * TOK],
                        in_=ao[jp * D + d * P:jp * D + (d + 1) * P, :])

            # ---- K^T [dk_all, 1024] and V [1024, dv] ----
            kT = atn.tile([P, KD, 4 * TOK], bf16, tag="kT", bufs=1)
            for m in range(KD):
                for c2 in range(2):
                    pk = ps.tile([P, 512], f32, tag="mm")
                    for k in range(KD):
                        nc.tensor.matmul(
                            pk, wkT[k][:, m * P:(m + 1) * P],
                            hT_all[:, k, c2 * 512:(c2 + 1) * 512],
                            start=(k == 0), stop=(k == KD - 1))
                    nc.scalar.activation(kT[:, m, c2 * 512:(c2 + 1) * 512], pk,
                                         AF.Identity, bias=bk_t[:, m:m + 1],
                                         scale=1.0)
            v_sb = []
            for tt in range(8):
                pv = ps.tile([P, D], f32, tag="mm")
                for k in range(KD):
                    nc.tensor.matmul(pv, hT_all[:, k, tt * P:(tt + 1) * P],
                                     wvT[k], start=(k == 0), stop=(k == KD - 1))
                vt_ = atn.tile([P, D], bf16, tag=f"v{tt}", bufs=1)
                nc.vector.tensor_copy(vt_, pv)
                v_sb.append(vt_)

            # ---- attention per head ----
            # key block kb=(jp, gp) <-> kT cols [256*jp+128*gp, +128), V tile 2*jp+gp
            oT = atn.tile([P, KD, TOK], bf16, tag="oT")
            sums = sm.tile([1, H * TOK], f32, tag="sums", bufs=2)
            for h in range(H):
                mt, bp = h // 2, 64 * (h % 2)
                kh = lambda col0, n: kT[bp:bp + DK, mt, col0:col0 + n]
                qh = qT[bp:bp + DK, mt, :]

                pT0 = atn.tile([P, 4, TOK], bf16, tag="pT0")
                pT1 = atn.tile([P, 4, P], bf16, tag="pT1")
                # scores + exp, kb pairs packed in one psum bank
                for jp in range(4):
                    # gp=0 -> q0(masked diag) + q1(live): [128, 256]
                    sc = pss.tile([P, TOK], f32, tag="sc")
                    nc.tensor.matmul(sc[:, 0:P], kh(256 * jp, P), qh[:, 0:P],
                                     start=True, stop=True)
                    nc.tensor.matmul(sc[:, P:TOK], kh(256 * jp, P), qh[:, P:TOK],
                                     start=True, stop=True)
                    nc.scalar.activation(pT0[:, jp, :], sc, AF.Exp)
                    nc.vector.tensor_mul(pT0[:, jp, 0:P], pT0[:, jp, 0:P],
                                         masks[:, jp, :])
                    # gp=1 -> q1 only, masked diag: [128, 128]
                    sc1 = pss.tile([P, P], f32, tag="sc")
                    nc.tensor.matmul(sc1, kh(256 * jp + P, P), qh[:, P:TOK],
                                     start=True, stop=True)
                    nc.scalar.activation(pT1[:, jp, :], sc1, AF.Exp)
                    nc.vector.tensor_mul(pT1[:, jp, :], pT1[:, jp, :],
                                         masks[:, jp, :])

                # denominators: ones-matmuls -> psum [1, 256]
                pd = pss.tile([1, TOK], f32, tag="pd", bufs=1)
                for jp in range(4):
                    nc.tensor.matmul(pd, ones_col, pT0[:, jp, :],
                                     start=(jp == 0), stop=False,
                                     skip_group_check=True)
                for jp in range(4):
                    nc.tensor.matmul(pd[:, P:TOK], ones_col, pT1[:, jp, :],
                                     start=False, stop=(jp == 3),
                                     skip_group_check=True)
                nc.scalar.copy(sums[:, h * TOK:(h + 1) * TOK], pd)

                # u^T accumulation [64, 256].  One bank: q0 region cols 0:128
                # (clears bank via start=True on its first mm), q1 region cols
                # 128:256 relies on has_written semantics (first write
                # overwrites, rest accumulate) so start=False throughout.
                pu = psu.tile([DK, TOK], f32, tag="pu")
                vh = lambda i: v_sb[i][:, h * DK:(h + 1) * DK]
                for jp in range(4):
                    nc.tensor.matmul(pu[:, 0:P], vh(2 * jp), pT0[:, jp, 0:P],
                                     start=(jp == 0), stop=(jp == 3),
                                     skip_group_check=True)
                q1_srcs = ([(2 * jp, pT0[:, jp, P:TOK]) for jp in range(4)] +
                           [(2 * jp + 1, pT1[:, jp, :]) for jp in range(4)])
                for i, (ti, src) in enumerate(q1_srcs):
                    nc.tensor.matmul(pu[:, P:TOK], vh(ti), src,
                                     start=False, stop=(i == 7),
                                     skip_group_check=True)

                # normalize: oT[h] = u^T * (1/sums) broadcast along partitions
                rec = sm.tile([1, TOK], f32, tag="rec", bufs=2)
                nc.vector.reciprocal(rec, sums[:, h * TOK:(h + 1) * TOK])
                recb = sm.tile([DK, TOK], f32, tag="recb", bufs=2)
                nc.gpsimd.partition_broadcast(recb, rec)
                nc.vector.tensor_mul(oT[bp:bp + DK, mt, :], pu, recb)
                nc.vector.tensor_scalar_add(
                    out=oT[bp:bp + DK, mt, :], in0=oT[bp:bp + DK, mt, :],
                    scalar1=bv_t[bp:bp + DK, mt:mt + 1])

            # ---- attention out-projection + residual + bo ----
            woT = load_w(W["wo"], KD, D, "wo")
            bo_t = wp.tile([P, D], f32, tag="bo")
            nc.sync.dma_start(out=bo_t, in_=W["bo"][:, :])
            for t in range(NT):
                py = ps.tile([P, D], f32, tag="mm")
                for k in range(KD):
                    nc.tensor.matmul(py, oT[:, k, t * P:(t + 1) * P], woT[k],
                                     start=(k == 0), stop=(k == KD - 1))
                xn = xp.tile([P, D], f32, tag=f"x{t}")
                nc.vector.tensor_add(xn, py, x_t[t])
                nc.vector.tensor_add(xn, xn, bo_t)
                x_t[t] = xn

            # ---- FFN ----
            h2 = layernorm(x_t, "h2_")
            h2T = transpose_own(h2, "h2T")
            w1T = load_w(W["w1"], KD, DFF, "w1")
            w2T = load_w(W["w2"], KF, D, "w2")
            b1_t = bias_tile(W["b1"], KF, "b1")
            b2_t = wp.tile([P, D], f32, tag="b2")
            nc.sync.dma_start(out=b2_t, in_=W["b2"][:, :])
            gT = act.tile([P, KF, TOK], bf16, tag="gT", bufs=1)
            for m in range(KF):
                pa = ps.tile([P, TOK], f32, tag="mm")
                for k in range(KD):
                    nc.tensor.matmul(pa, w1T[k][:, m * P:(m + 1) * P],
                                     h2T[:, k, :],
                                     start=(k == 0), stop=(k == KD - 1))
                nc.scalar.activation(gT[:, m, :], pa, AF.Gelu,
                                     bias=b1_t[:, m:m + 1], scale=1.0)
            for t in range(NT):
                pz = ps.tile([P, D], f32, tag="mm")
                for k in range(KF):
                    nc.tensor.matmul(pz, gT[:, k, t * P:(t + 1) * P], w2T[k],
                                     start=(k == 0), stop=(k == KF - 1))
                xn = xp.tile([P, D], f32, tag=f"x{t}")
                nc.vector.tensor_add(xn, pz, x_t[t])
                nc.vector.tensor_add(xn, xn, b2_t)
                x_t[t] = xn
                if debug:
                    nc.sync.dma_start(out=xdbg[l, t * P:(t + 1) * P, :], in_=xn)

        # ---- final LN + 8-way AllGather + lm_head ----
        hf = layernorm(x_t, "hf_")
        hfT = transpose_own(hf, "hfT")
        nc.sync.dma_start(
            out=bass.AP(tensor=af_i, offset=0,
                        ap=[[TOK, P], [P * TOK, KD], [1, TOK]]),
            in_=hfT)
        nc.gpsimd.collective_compute(
            "AllGather", Alu.bypass, replica_groups=GROUPS_ALL,
            ins=[af_i.ap().opt()], outs=[af_o.ap().opt()])

        xfT = act.tile([P, KD, NC * TOK], bf16, tag="xfT", bufs=1)
        for rk in range(NC):
            for d in range(KD):
                nc.sync.dma_start(
                    out=xfT[:, d, rk * TOK:(rk + 1) * TOK],
                    in_=af_o[rk * D + d * P:rk * D + (d + 1) * P, :])

        bh_t = bias_tile(bhead, vt, "bh")
        NCHUNK = (B * S) // 512
        MC = 10
        for m0 in range(0, vt, MC):
            mn = min(MC, vt - m0)
            whc = []
            for k in range(KD):
                wt = whp.tile([P, MC * P], bf16, tag=f"wh{k}", bufs=2)
                nc.sync.dma_start(
                    out=wt[:, :mn * P],
                    in_=whead[k * P:(k + 1) * P, m0 * P:(m0 + mn) * P])
                whc.append(wt)
            for mi in range(mn):
                m = m0 + mi
                for c2 in range(NCHUNK):
                    pl = ps.tile([P, 512], f32, tag="mm")
                    for k in range(KD):
                        nc.tensor.matmul(
                            pl, whc[k][:, mi * P:(mi + 1) * P],
                            xfT[:, k, c2 * 512:(c2 + 1) * 512],
                            start=(k == 0), stop=(k == KD - 1))
                    lo = act.tile([P, 512], f32, tag=f"lo{(m + c2) % 2}")
                    if (m + c2) % 2 == 0:
                        nc.scalar.activation(lo, pl, AF.Identity,
                                             bias=bh_t[:, m:m + 1], scale=1.0)
                    else:
                        nc.vector.tensor_scalar(out=lo, in0=pl,
                                                scalar1=bh_t[:, m:m + 1],
                                                scalar2=None, op0=Alu.add)
                    nc.sync.dma_start(
                        out=logits_out[m * P:(m + 1) * P,
                                       c2 * 512:(c2 + 1) * 512],
                        in_=lo)

    nc.compile()
    return nc


# --------------------------------------------------------------------------
# host side
# --------------------------------------------------------------------------

def host_prep(inputs, n_layers=L, vt=VT_FULL):
    """per-core in_maps + token permutation metadata"""
    emb = _f32(inputs["embedding"])
    pos = _f32(inputs["pos_embedding"])[0, :S]
    tokens = np.asarray(inputs["tokens"]).astype(np.int64)

    in_maps = []
    g1 = _f32(inputs["ln1_g"]); b1l = _f32(inputs["ln1_b"])
    g2 = _f32(inputs["ln2_g"]); b2l = _f32(inputs["ln2_b"])
    gf = _f32(inputs["lnf_g"]); bfl = _f32(inputs["lnf_b"])

    shared = {}
    for l in range(n_layers):
        Wq, Wk, Wv, Wo = (_f32(inputs[k][l]) for k in ["Wq", "Wk", "Wv", "Wo"])
        W1, W2 = _f32(inputs["W1"][l]), _f32(inputs["W2"][l])
        bq, bk, bv, bo = (_f32(inputs[k][l]) for k in ["bq", "bk", "bv", "bo"])
        b1, b2 = _f32(inputs["b1"][l]), _f32(inputs["b2"][l])
        shared[f"wq{l}"] = _bf(((Wq * g1[l]) / 8.0).T)
        shared[f"wk{l}"] = _bf((Wk * g1[l]).T)
        shared[f"wv{l}"] = _bf((Wv * g1[l]).T)
        shared[f"wo{l}"] = _bf(Wo.T)
        shared[f"w1{l}"] = _bf((W1 * g2[l]).T)
        shared[f"w2{l}"] = _bf(W2.T)
        shared[f"bq{l}"] = _f32((bq + Wq @ b1l[l]) / 8.0)
        shared[f"bk{l}"] = _f32(bk + Wk @ b1l[l])
        shared[f"bv{l}"] = _f32(bv + Wv @ b1l[l])
        shared[f"bo{l}"] = np.broadcast_to(bo, (P, D)).copy()
        shared[f"b1{l}"] = _f32(b1 + W1 @ b2l[l])
        shared[f"b2{l}"] = np.broadcast_to(b2, (P, D)).copy()

    Whead = _f32(inputs["Whead"]); bh = _f32(inputs["bhead"])
    Whead_eff = Whead * gf
    bh_eff = bh + Whead @ bfl

    for c in range(NC):
        b, j = c // 4, c % 4
        m = {"x0": np.zeros((TOK, D), np.float32)}
        for g in range(NT):
            t_ids = 512 * g + 4 * np.arange(P) + j
            m["x0"][g * P:(g + 1) * P] = emb[tokens[b, t_ids]] + pos[t_ids]
        # masks[jp][r_k, r_q]: live iff 4*r_k + jp <= 4*r_q + j
        mk = np.zeros((4, P, P), np.float32)
        for jp in range(4):
            rk = np.arange(P)[:, None]; rq = np.arange(P)[None, :]
            mk[jp] = (rk <= rq - (1 if jp > j else 0)).astype(np.float32)
        m["masks"] = _bf(mk)
        v0 = sum(VS[:c])
        wslice = np.zeros((D, vt * P), np.float32)
        bslice = np.zeros((vt * P,), np.float32)
        n = min(VS[c], vt * P)
        wslice[:, :n] = Whead_eff.T[:, v0:v0 + n]
        bslice[:n] = bh_eff[v0:v0 + n]
        m["whead"] = _bf(wslice)
        m["bhead"] = _f32(bslice)
        m.update(shared)
        in_maps.append(m)
    return in_maps


def assemble(results, vt=VT_FULL):
    """per-core logitsT [vt*P, 2048] -> [B, S, V] with token un-permutation"""
    gam = np.arange(NC * TOK)
    cp = gam // TOK; w = gam % TOK
    gp = w // P; rp = w % P
    bb = cp // 4; jj = cp % 4
    t = 512 * gp + 4 * rp + jj
    rows = bb * S + t          # gathered col -> flat token row
    out = np.empty((B * S, V), np.float32)
    for c in range(NC):
        v0 = sum(VS[:c])
        lt = results[c]["logitsT"][:VS[c]]     # [vs, 2048]
        out[rows, v0:v0 + VS[c]] = lt.T
    return out.reshape(B, S, V)


_CACHE = {}


def kernel(**inputs):
    key = ("full", L, VT_FULL)
    if key not in _CACHE:
        _CACHE[key] = build(L, VT_FULL, debug=False)
    nc = _CACHE[key]
    in_maps = host_prep(inputs, L, VT_FULL)
    res = run_bass_kernel_spmd(nc, in_maps, list(range(NC)))
    return assemble(res.results, VT_FULL)
